# revision 42
# baseline (speedup 1.0000x reference)
"""Trainium2 Bass kernel for nn_Enhancement_77309412162.

Math reduction (from the reference):
  theta[b,n] = sum_c x[b,c,n]*theta_w[c] + theta_b        (per-sample matvec)
  g[b,n]     = sum_c x[b,c,n]*g_w[c] + g_b
  phi1[b,n]  = sum_c x1[b,c,n]*phi_w[c] + phi_b
  phi2[b,n]  = sum_c x2[b,c,n]*phi_w[c] + phi_b
  The (N,N) affinity matrices are rank-1, so
  y[b,n] = s_b * theta[b,n],  s_b = (b/N)*(a_c*<phi1,g> + (1-a_c)*<phi2,g>)
  wy[b,c,n] = W_w[c]*t[b,n] + W_b[c],  t = s_b*theta_b
  BN over (B,H,W):  mean[c] = W_w[c]*mu + W_b[c],  var[c] = W_w[c]^2*var_t
  where mu/var_t are the global scalar mean/var of t over all (b,n).
  out[b,c,n] = x[b,c,n] + alpha[c]*(t[b,n]-mu) + bn_b[c]
  with alpha[c] = bn_w[c]*W_w[c]/sqrt(W_w[c]^2*var_t + 1e-5).

Sharding: batch-parallel, one sample per core (B=8, 8 cores). The only
cross-core data is an AllGather of [sum(t), sum(t^2)] (8 bytes/core).

Implementation notes:
- theta+g computed in one M=2 matmul pass over x; phi passes use a
  duplicated-weight M=2 lhsT so phi lands on partition 1 next to g.
- Per-sample scalar partials are placed on the partitions where they are
  computable, then a K=2 ones-matmul both sums across the two partitions
  and broadcasts the result to all 128 partitions, so the whole scalar
  chain runs replicated with no cross-partition moves.
- The t-broadcast (ones (x) theta) runs early into SBUF so the post-
  collective tail is just scale/bias + residual add + store.
"""

import os
import numpy as np

B, C, H, W = 8, 512, 48, 48
N = H * W            # 2304
P = 128
J = C // P           # 4 channel chunks
NCHUNKS = [(0, 512), (512, 512), (1024, 512), (1536, 512), (2048, 256)]
NC5 = len(NCHUNKS)
NCORES = 8
BN_COUNT = float(B * N)

_cache = {}


def _rdma_butterfly(tc, pool, upair, rsems, f32):
    """Allreduce-add of the (128,2) `upair` across the 8 cores of the chip
    via 3 rounds of XOR-partner exchange over direct SBUF->SBUF remote DMA.

    Round r: send my running sum to tpb (mine XOR 2^r) with a relative-dest
    broadcast descriptor (slot 2^r keeps cross-die traffic on D2D-capable
    engines), wait for my partner's value (remote sem +2: two DMA lanes per
    dest at n_dests=8), add. All on GpSimd so program order is execution
    order inside the critical section.
    """
    nc = tc.nc
    P = 128
    ucur = upair
    rts = [pool.tile([P, 2], f32, name=f"rt{r}") for r in range(3)]
    uns = [pool.tile([P, 2], f32, name=f"un{r}") for r in range(3)]
    lsem, asem = rsems[3], rsems[4]
    with tc.tile_critical():
        for r in range(3):
            delta = 1 << r
            rdests = [None] * 8
            rdests[delta] = (0, delta)
            if r > 0:
                nc.gpsimd.wait_ge(asem, r)
            nc.gpsimd.remote_dma_broadcast(
                out_ap=rts[r][:], in_ap=ucur[:],
                remote_sem=rsems[r], local_sem=lsem,
                rdests=rdests)
            nc.gpsimd.trigger_dma(1)
            nc.gpsimd.wait_ge(rsems[r], 2)
            nc.gpsimd.tensor_add(uns[r], ucur, rts[r]).then_inc(asem, 1)
            ucur = uns[r]
    return ucur


def _build_nc():
    import concourse.bass as bass
    import concourse.bacc as bacc
    import concourse.tile as tile
    from concourse import mybir
    from contextlib import ExitStack

    f32 = mybir.dt.float32
    f32r = mybir.dt.float32r
    # 0: all-fp32; 1: all-fp32r; 2: fp32r for the phi passes only (theta/g
    # stay exact; phi errors only perturb the scalar s, which BN cancels)
    f32r_mode = int(os.environ.get("KERNEL_FP32R", "2"))
    use_f32r = f32r_mode == 1
    mm_dt = f32r if use_f32r else f32
    phi_dt = f32r if f32r_mode in (1, 2) else f32
    Alu = mybir.AluOpType
    Act = mybir.ActivationFunctionType
    AxX = mybir.AxisListType.X

    nc = bacc.Bacc("TRN2", target_bir_lowering=False, debug=False,
                   enable_asserts=False, num_devices=NCORES)

    x_d = nc.dram_tensor("x", [C, N], f32, kind="ExternalInput").ap()
    x1_d = nc.dram_tensor("x1", [C, N], f32, kind="ExternalInput").ap()
    x2_d = nc.dram_tensor("x2", [C, N], f32, kind="ExternalInput").ap()
    thw_d = nc.dram_tensor("theta_w", [C], f32, kind="ExternalInput").ap()
    gw_d = nc.dram_tensor("g_w", [C], f32, kind="ExternalInput").ap()
    phw_d = nc.dram_tensor("phi_w", [C], f32, kind="ExternalInput").ap()
    thb_d = nc.dram_tensor("theta_b", [1], f32, kind="ExternalInput").ap()
    gb_d = nc.dram_tensor("g_b", [1], f32, kind="ExternalInput").ap()
    phb_d = nc.dram_tensor("phi_b", [1], f32, kind="ExternalInput").ap()
    ww_d = nc.dram_tensor("W_w", [C], f32, kind="ExternalInput").ap()
    bnw_d = nc.dram_tensor("bn_w", [C], f32, kind="ExternalInput").ap()
    bnb_d = nc.dram_tensor("bn_b", [C], f32, kind="ExternalInput").ap()
    a_d = nc.dram_tensor("a", [1], f32, kind="ExternalInput").ap()
    b_d = nc.dram_tensor("b", [1], f32, kind="ExternalInput").ap()
    out_d = nc.dram_tensor("out", [C, N], f32, kind="ExternalOutput").ap()

    with tile.TileContext(nc) as tc, ExitStack() as ctx:
        singles = ctx.enter_context(tc.tile_pool(name="singles", bufs=1))
        xpool = ctx.enter_context(tc.tile_pool(name="xpool", bufs=J))
        stream = ctx.enter_context(tc.tile_pool(name="stream", bufs=7))
        tmps = ctx.enter_context(tc.tile_pool(name="tmps", bufs=2))
        scr = ctx.enter_context(tc.tile_pool(name="scr", bufs=2))
        psproj = ctx.enter_context(tc.tile_pool(name="psproj", bufs=4, space="PSUM"))
        psbc = ctx.enter_context(tc.tile_pool(name="psbc", bufs=2, space="PSUM"))
        psr = ctx.enter_context(tc.tile_pool(name="psr", bufs=1, space="PSUM"))
        dram = ctx.enter_context(tc.tile_pool(name="dram", bufs=1, space="DRAM"))

        # ---- small constant loads (SWDGE so they don't queue behind bulk;
        # SWDGE also casts fp32 -> fp32r in flight when enabled) ----
        wxt = singles.tile([P, J, 2], mm_dt, name="wxt")     # [theta_w | g_w]
        nc.gpsimd.dma_start(out=wxt[:, :, 0],
                            in_=thw_d.rearrange("(j p) -> p j", p=P))
        nc.gpsimd.dma_start(out=wxt[:, :, 1],
                            in_=gw_d.rearrange("(j p) -> p j", p=P))

        wpt = singles.tile([P, J, 2], f32, name="wpt")      # [phi_w | phi_w]
        nc.gpsimd.dma_start(out=wpt[:, :, 0],
                            in_=phw_d.rearrange("(j p) -> p j", p=P))
        nc.gpsimd.dma_start(out=wpt[:, :, 1],
                            in_=phw_d.rearrange("(j p) -> p j", p=P))

        def load_pj(ap_d, nm):
            t = singles.tile([P, J], f32, name=nm)
            nc.gpsimd.dma_start(out=t, in_=ap_d.rearrange("(j p) -> p j", p=P))
            return t

        phw_t = load_pj(phw_d, "phw_t")
        ww = load_pj(ww_d, "ww")
        bnw = load_pj(bnw_d, "bnw")
        bnb = load_pj(bnb_d, "bnb")

        thgb = singles.tile([2, 1], f32, name="thgb")   # row0 theta_b, row1 g_b
        nc.gpsimd.dma_start(out=thgb[0:1, :], in_=thb_d[None, :])
        nc.gpsimd.dma_start(out=thgb[1:2, :], in_=gb_d[None, :])
        phb2 = singles.tile([2, 1], f32, name="phb2")
        nc.gpsimd.dma_start(out=phb2, in_=bass.AP(tensor=phb_d.tensor,
                                                  offset=phb_d.offset,
                                                  ap=[[0, 2], [1, 1]]))
        phb128 = singles.tile([P, 1], f32, name="phb128")
        nc.gpsimd.dma_start(out=phb128, in_=bass.AP(tensor=phb_d.tensor,
                                                    offset=phb_d.offset,
                                                    ap=[[0, P], [1, 1]]))
        av128 = singles.tile([P, 1], f32, name="av128")
        nc.gpsimd.dma_start(out=av128, in_=bass.AP(tensor=a_d.tensor,
                                                   offset=a_d.offset,
                                                   ap=[[0, P], [1, 1]]))
        bv128 = singles.tile([P, 1], f32, name="bv128")
        nc.gpsimd.dma_start(out=bv128, in_=bass.AP(tensor=b_d.tensor,
                                                   offset=b_d.offset,
                                                   ap=[[0, P], [1, 1]]))

        rsems = None
        if os.environ.get("KERNEL_CC", "ncfw") == "rdma":
            # semaphores for the remote-DMA butterfly; same program on every
            # core => same indices everywhere. Cleared here, long before any
            # peer's butterfly sends can land, because the hardware does not
            # zero semaphores between runs.
            rsems = [nc.alloc_semaphore(name=f"rdma_sem{i}") for i in range(5)]
            for s in rsems[:3] + rsems[4:]:
                nc.gpsimd.sem_clear(s)
        if int(os.environ.get("KERNEL_CC_WARM", "0")):
            # dummy collective issued early: absorbs the ncfw first-use cost
            # under the input stream so the real one later is cheap
            warm_in = dram.tile([1, 2], f32, name="warm_in")
            warm_out = dram.tile([1, 2 * NCORES], f32, name="warm_out")
            nc.gpsimd.collective_compute(
                "AllGather", Alu.bypass,
                replica_groups=[list(range(NCORES))],
                ins=[warm_in.opt()], outs=[warm_out.opt()],
            )

        ones1 = singles.tile([1, P], f32, name="ones1")
        nc.vector.memset(ones1, 1.0)
        # selector lhsTs: sel0 broadcasts partition-0 values to all 128
        # partitions, sel1 broadcasts partition-1 values
        sel0 = singles.tile([2, P], f32, name="sel0")
        nc.vector.memset(sel0, 0.0)
        nc.vector.memset(sel0[0:1, :], 1.0)
        sel1 = singles.tile([2, P], f32, name="sel1")
        nc.vector.memset(sel1, 1.0)
        nc.vector.tensor_sub(sel1, sel1, sel0)
        ones128 = singles.tile([P, P], f32, name="ones128")
        nc.vector.memset(ones128, 1.0)
        # partials, always written as (2,.) pairs; partner row is ignored:
        # col0 row0=A=sum(theta) row1=C=sum(g), col1 row0=B=sum(theta^2),
        # col2 row1=d2=<phi2,g> (both with biases folded in)
        PT = singles.tile([2, 3], f32, name="PT")

        # ---- bulk input: x stays resident (needed again for the residual) ---
        # Bulk loads use only the two HWDGE rings: GpSimd must stay free so
        # the warm-up collective is issued immediately (a late warm-up
        # serializes in front of the real collective and doubles its cost).
        _dge = [nc.sync, nc.scalar]
        _dge_i = [0]

        def bulk_dma(out, in_):
            eng = _dge[_dge_i[0] % 2]
            _dge_i[0] += 1
            eng.dma_start(out=out, in_=in_)

        x_tiles = []
        for j in range(J):
            xt = xpool.tile([P, N], f32, name="xt")
            bulk_dma(xt, x_d[j * P:(j + 1) * P, :])
            x_tiles.append(xt)
        if use_f32r:
            # fp32r matmuls need explicitly-rounded inputs; x itself must
            # stay fp32 for the residual add, so round a copy on ScalarE.
            xr_tiles = []
            for j in range(J):
                xr = xpool.tile([P, N], f32r, name="xr")
                nc.scalar.activation(out=xr, in_=x_tiles[j], func=Act.Copy)
                xr_tiles.append(xr)
        else:
            xr_tiles = x_tiles

        thg = singles.tile([2, N], f32, name="thg")     # row0 theta, row1 g

        def project(dst, w_pj, b_21, src_tiles):
            # dst[{0,1}, n] = sum_c src[c, n] * w[c, {0,1}] + b
            for (n0, nsz) in NCHUNKS:
                ps = psproj.tile([2, 512], f32, name="ps_proj")
                for j in range(J):
                    nc.tensor.matmul(ps[:, :nsz], lhsT=w_pj[:, j, :],
                                     rhs=src_tiles[j][:, n0:n0 + nsz],
                                     start=(j == 0), stop=(j == J - 1))
                nc.scalar.activation(out=dst[:, n0:n0 + nsz], in_=ps[:, :nsz],
                                     func=Act.Identity, bias=b_21, scale=1.0)

        project(thg, wxt, thgb, xr_tiles)

        # t-broadcast: theta AND g replicated to all 128 partitions via a
        # DRAM bounce + stride-0-partition read (frees ~20k cold PE cycles
        # vs doing it with ones-matmuls). g_bc feeds the VectorE dots below;
        # theta_bc feeds the final pass.
        theta_bc = singles.tile([P, N], f32, name="theta_bc")
        g_bc = singles.tile([P, N], f32, name="g_bc")
        thg_dram = dram.tile([2, N], f32, name="thg_dram")
        nc.sync.dma_start(out=thg_dram, in_=thg)
        nc.scalar.dma_start(out=g_bc, in_=bass.AP(tensor=thg_dram.tensor,
                                                  offset=thg_dram.offset + N,
                                                  ap=[[0, P], [1, N]]))
        nc.sync.dma_start(out=theta_bc, in_=bass.AP(tensor=thg_dram.tensor,
                                                    offset=thg_dram.offset,
                                                    ap=[[0, P], [1, N]]))

        # A = sum(theta) & C = sum(g) (accum rows 0/1), B = sum(theta^2)
        sq_scr = scr.tile([2, N], f32, name="sq_scr")
        nc.scalar.activation(out=sq_scr, in_=thg, func=Act.Identity,
                             accum_out=PT[:, 0:1])
        nc.scalar.activation(out=sq_scr, in_=thg, func=Act.Square,
                             accum_out=PT[:, 1:2])

        def load_stream(src_d, j):
            st = stream.tile([P, N], f32, name="stream_t")
            bulk_dma(st, src_d[j * P:(j + 1) * P, :])
            return st

        # phi dots without materializing phi: d_raw = <phi_w, X @ g> with the
        # per-channel weighted row sums X @ g computed on VectorE against the
        # broadcast g (exact fp32; PE stays free for theta/g + broadcasts).
        rjk = singles.tile([P, J, NC5], f32, name="rjk")

        x1_tiles = [load_stream(x1_d, j) for j in range(J)]
        for j in range(J):
            for k, (n0, nsz) in enumerate(NCHUNKS):
                ms = scr.tile([P, 512], f32, name="mul_scr")
                nc.vector.tensor_mul(ms[:, :nsz],
                                     x1_tiles[j][:, n0:n0 + nsz],
                                     g_bc[:, n0:n0 + nsz])
                nc.vector.tensor_reduce(rjk[:, j, k:k + 1],
                                        ms[:, :nsz], axis=AxX, op=Alu.add)

        # phi2 via PE projection (PE is free once theta/g is done and this
        # needs no g_bc, so it overlaps the x1 DVE dots), then a (2,N)
        # mult+reduce gives d2 = <phi2,g> with both biases included.
        phi2 = singles.tile([2, N], f32, name="phi2")
        x2_tiles = [load_stream(x2_d, j) for j in range(J)]
        project(phi2, wpt, phb2, x2_tiles)
        # d2 per chunk so only the last 256-col piece sits on the tail
        d2k = singles.tile([2, NC5], f32, name="d2k")
        for k, (n0, nsz) in enumerate(NCHUNKS):
            d2_scr = scr.tile([2, 512], f32, name="d2_scr")
            nc.vector.tensor_mul(d2_scr[:, :nsz], phi2[:, n0:n0 + nsz],
                                 thg[:, n0:n0 + nsz])
            nc.vector.tensor_reduce(d2k[:, k:k + 1], d2_scr[:, :nsz],
                                    axis=AxX, op=Alu.add)
        nc.vector.tensor_reduce(PT[:, 2:3], d2k, axis=AxX, op=Alu.add)

        # collapse chunks, weight by phi_w per channel, sum over channels;
        # the ones128 matmul sums the 128 partitions AND broadcasts back
        rj = singles.tile([P, J], f32, name="rj")
        nc.vector.tensor_reduce(rj, rjk, axis=AxX, op=Alu.add)
        nc.vector.tensor_mul(rj, rj, phw_t)
        dp = singles.tile([P, 1], f32, name="dp")
        nc.vector.tensor_reduce(dp, rj, axis=AxX, op=Alu.add)
        pr2 = psr.tile([P, 1], f32, name="pr2")
        nc.tensor.matmul(pr2, lhsT=ones128, rhs=dp, start=True, stop=True)
        draw = singles.tile([P, 1], f32, name="draw")
        nc.scalar.activation(out=draw, in_=pr2, func=Act.Copy)

        # broadcast the (2,3) partials rows to all 128 partitions
        pr = psr.tile([P, 6], f32, name="pr")
        nc.tensor.matmul(pr[:, 0:3], lhsT=sel0, rhs=PT, start=True, stop=True)
        nc.tensor.matmul(pr[:, 3:6], lhsT=sel1, rhs=PT, start=True, stop=True)
        r_sb = singles.tile([P, 6], f32, name="r_sb")
        nc.scalar.activation(out=r_sb, in_=pr, func=Act.Copy)
        A_ = r_sb[:, 0:1]
        B_ = r_sb[:, 1:2]
        C_ = r_sb[:, 3:4]
        d2_ = r_sb[:, 5:6]

        # d1 = draw + phi_b*C (g carries its own bias), then
        # s = (b/N) * (d2 + a_c*(d1-d2)); u1 = s*A; u2 = s^2*B  (replicated)
        pbc_ = singles.tile([P, 1], f32, name="pbc_")
        nc.vector.tensor_mul(pbc_, phb128, C_)
        d1_ = singles.tile([P, 1], f32, name="d1_")
        nc.vector.tensor_add(d1_, draw, pbc_)

        ac = singles.tile([P, 1], f32, name="ac")
        nc.vector.tensor_scalar(ac, av128, 0.0, 1.0, op0=Alu.max, op1=Alu.min)
        sv = singles.tile([P, 1], f32, name="sv")
        nc.vector.tensor_sub(sv, d1_, d2_)
        nc.vector.tensor_mul(sv, sv, ac)
        nc.vector.tensor_add(sv, sv, d2_)
        nc.vector.tensor_mul(sv, sv, bv128)
        nc.vector.tensor_scalar_mul(sv, sv, 1.0 / float(N))
        s2v = singles.tile([P, 1], f32, name="s2v")
        nc.vector.tensor_mul(s2v, sv, sv)
        upair = singles.tile([P, 2], f32, name="upair")
        nc.vector.tensor_mul(upair[:, 0:1], sv, A_)
        nc.vector.tensor_mul(upair[:, 1:2], s2v, B_)

        # ---- 8-byte-per-core allreduce across the 8 cores ----
        cc_mode = os.environ.get("KERNEL_CC", "ncfw")
        if cc_mode == "rdma":
            # 3-round XOR-butterfly over direct SBUF->SBUF remote DMA.
            # Relative dests (delta-tpb = 1,2,4) make the program fully SPMD-
            # symmetric: no partition id or routing id needed. Everything runs
            # on GpSimd inside one critical section for strict ordering.
            uu = _rdma_butterfly(tc, singles, upair, rsems, f32)
        else:
            cc_in = dram.tile([1, 2], f32, name="cc_in")
            cc_out = dram.tile([1, 2 * NCORES], f32, name="cc_out")
            nc.gpsimd.dma_start(out=cc_in, in_=upair[0:1, :])
            nc.gpsimd.collective_compute(
                "AllGather", Alu.bypass,
                replica_groups=[list(range(NCORES))],
                ins=[cc_in.opt()], outs=[cc_out.opt()],
            )
            bcG = singles.tile([P, 2 * NCORES], f32, name="bcG")
            nc.gpsimd.dma_start(out=bcG, in_=bass.AP(tensor=cc_out.tensor,
                                                     offset=cc_out.offset,
                                                     ap=[[0, P], [1, 2 * NCORES]]))
            uu = singles.tile([P, 2], f32, name="uu")
            nc.vector.tensor_reduce(uu, bcG.rearrange("p (r i) -> p i r", i=2),
                                    axis=AxX, op=Alu.add)

        # global stats -> per-channel scale/bias (column j = channels j*128+p)
        muv = singles.tile([P, 1], f32, name="muv")
        nc.vector.tensor_scalar_mul(muv, uu[:, 0:1], 1.0 / BN_COUNT)
        varv = singles.tile([P, 1], f32, name="varv")
        nc.vector.tensor_scalar_mul(varv, uu[:, 1:2], 1.0 / BN_COUNT)
        musq = singles.tile([P, 1], f32, name="musq")
        nc.vector.tensor_mul(musq, muv, muv)
        nc.vector.tensor_sub(varv, varv, musq)
        dv = singles.tile([P, J], f32, name="dv")
        nc.vector.tensor_mul(dv, ww, ww)
        nc.vector.tensor_scalar(dv, dv, varv, 1e-5, op0=Alu.mult, op1=Alu.add)
        nc.scalar.activation(out=dv, in_=dv, func=Act.Sqrt)
        rst = singles.tile([P, J], f32, name="rst")
        nc.vector.reciprocal(rst, dv)
        alpha = singles.tile([P, J], f32, name="alpha")
        nc.vector.tensor_mul(alpha, bnw, ww)
        nc.vector.tensor_mul(alpha, alpha, rst)
        scale2 = singles.tile([P, J], f32, name="scale2")
        nc.vector.tensor_scalar(scale2, alpha, sv, None, op0=Alu.mult)
        bias2 = singles.tile([P, J], f32, name="bias2")
        nc.vector.tensor_scalar(bias2, alpha, muv, None, op0=Alu.mult)
        nc.vector.tensor_sub(bias2, bnb, bias2)

        # out = x + scale2[c]*theta_bc + bias2[c]; one full-row ACT op per
        # channel block (ACT per-op dispatch is ~1.5us, so fewer bigger ops),
        # adds and stores split in halves so the DMA drains early.
        HALF = N // 2
        for j in range(J):
            tmp = tmps.tile([P, N], f32, name="tmp")
            nc.scalar.activation(out=tmp, in_=theta_bc, func=Act.Identity,
                                 scale=scale2[:, j:j + 1],
                                 bias=bias2[:, j:j + 1])
            for h in range(2):
                sl = slice(h * HALF, (h + 1) * HALF)
                nc.vector.tensor_add(x_tiles[j][:, sl], x_tiles[j][:, sl],
                                     tmp[:, sl])
                bulk_dma(out_d[j * P:(j + 1) * P, sl], x_tiles[j][:, sl])

    nc.compile()
    return nc


def kernel(**inputs):
    from concourse import bass_utils

    nc = _cache.get("nc")
    if nc is None:
        nc = _build_nc()
        _cache["nc"] = nc

    def f32c(a):
        return np.ascontiguousarray(np.asarray(a, dtype=np.float32))

    xs = f32c(inputs["x"]).reshape(B, C, N)
    x1s = f32c(inputs["x1"]).reshape(B, C, N)
    x2s = f32c(inputs["x2"]).reshape(B, C, N)
    shared = {
        "theta_w": f32c(inputs["theta_w"]),
        "g_w": f32c(inputs["g_w"]),
        "phi_w": f32c(inputs["phi_w"]),
        "theta_b": f32c(inputs["theta_b"]),
        "g_b": f32c(inputs["g_b"]),
        "phi_b": f32c(inputs["phi_b"]),
        "W_w": f32c(inputs["W_w"]),
        "bn_w": f32c(inputs["bn_w"]),
        "bn_b": f32c(inputs["bn_b"]),
        "a": f32c(inputs["a"]),
        "b": f32c(inputs["b"]),
    }
    in_maps = [
        {"x": xs[c], "x1": x1s[c], "x2": x2s[c], **shared}
        for c in range(NCORES)
    ]
    res = bass_utils.run_bass_kernel_spmd(
        nc, in_maps, core_ids=list(range(NCORES)),
        trace=bool(os.environ.get("BASS_TRACE")),
        tmpdir=os.environ.get("KERNEL_TMPDIR") or None,
    )
    _cache["last_results"] = res
    out = np.stack([res.results[c]["out"] for c in range(NCORES)], axis=0)
    return out.reshape(B, C, H, W)
